# revision 1
# baseline (speedup 1.0000x reference)
"""GAT layer on 8 Trainium2 NeuronCores (Bass/Tile), edge-parallel dst-sharded.

Self-contained: host preprocesses the graph (self-loops, dst-shard, bucket
sort, uniform bucket cap), the device program computes Wh/attention tables,
AllGathers the [Wh|el] table, then per 128-node bucket: indirect-DMA gathers
of table rows by src and er rows by dst, scores -> leaky-relu -> exp, and a
one-hot scatter matmul accumulated in PSUM, normalized and written out.
"""
import sys

for _p in ("/opt/trn_rl_repo",):
    if _p not in sys.path:
        sys.path.insert(0, _p)

import numpy as np
import ml_dtypes

import concourse.bass as bass
import concourse.tile as tile
from concourse import mybir
from concourse.bass import IndirectOffsetOnAxis
from concourse.bass_utils import run_bass_kernel_spmd

BF16 = ml_dtypes.bfloat16

N = 50000
E = 800000
IN = 256
H = 8
C = 32
HC = H * C            # 256
NC = 8
NPC = N // NC         # 6250 nodes per core
BUCKET = 128
NBUCK = (NPC + BUCKET - 1) // BUCKET   # 49
XT_PAD = NBUCK * 128                   # 6272
PAY = HC + H          # 264: table row [Wh(256) | el(8)]
P1COLS = HC + 2 * H   # 272: phase-1 matmul out [Wh | el | er]
NEG = 0.2
EPS = 1e-16
SC_BUCKETS = 2        # buckets per gather super-chunk

# walrus in this container caps sync waits per instruction at 1; hoist excess
# onto same-engine NoOps.
_waitfix_ctr = [0]


def _split_excess_waits(nc, max_waits=1):
    n_fixed = 0
    for fn in nc.m.functions:
        for bb in fn.blocks:
            insts = bb.instructions
            out = []
            for ins in insts:
                si = ins.sync_info
                waits = list(si.on_wait) if si is not None and si.on_wait else []
                if len(waits) > max_waits:
                    keep = waits[-max_waits:]
                    extra = waits[:-max_waits]
                    for i in range(0, len(extra), max_waits):
                        grp = extra[i:i + max_waits]
                        _waitfix_ctr[0] += 1
                        nop = mybir.InstNoOp(
                            name=f"I-waitfix-{_waitfix_ctr[0]}", ins=[], outs=[])
                        nop.engine = ins.engine
                        nop.sync_info = mybir.SyncInfo(on_wait=grp, on_update=[])
                        nc.register_instruction(nop)
                        out.append(nop)
                    si.on_wait = keep
                    n_fixed += 1
                out.append(ins)
            if len(out) != len(insts):
                bb.instructions = out
    return n_fixed


def _host_prep(x, edge_index, W, a_left, a_right):
    src = np.concatenate([edge_index[0], np.arange(N, dtype=np.int64)])
    dst = np.concatenate([edge_index[1], np.arange(N, dtype=np.int64)])
    src = src.astype(np.int64)
    dst = dst.astype(np.int64)

    # fold attention vectors through W:  [el|er] = x @ (W.T @ A)
    A = np.zeros((HC, 2 * H), np.float32)
    for h in range(H):
        A[h * C:(h + 1) * C, h] = a_left[h]
        A[h * C:(h + 1) * C, H + h] = a_right[h]
    B = (W.T.astype(np.float64) @ A.astype(np.float64)).astype(np.float32)
    wtb = np.concatenate([W.T.astype(np.float32), B], axis=1).astype(BF16)  # [256, 272]

    core = dst // NPC

    # uniform per-bucket cap across every core (SPMD: one program)
    gmax = 0
    for c in range(NC):
        m = core == c
        b = (dst[m] - c * NPC) // BUCKET
        cnt = np.bincount(b, minlength=NBUCK)
        gmax = max(gmax, int(cnt.max()))
    g_cap = ((gmax + 127) // 128) * 128
    nblkb = g_cap // 128          # blocks per bucket
    nblk = NBUCK * nblkb          # blocks per core

    goff = np.zeros((NC, 128, nblk), np.int32)
    eroff = np.zeros((NC, 128, nblk), np.int32)
    dloc = np.full((NC, 128, nblk), 200.0, BF16)
    xT = np.zeros((NC, IN, XT_PAD), BF16)

    for c in range(NC):
        m = core == c
        s_c = src[m]
        d_c = dst[m]
        dl = d_c - c * NPC
        b_c = dl // BUCKET
        order = np.lexsort((s_c, b_c))
        s_c, dl, b_c = s_c[order], dl[order], b_c[order]

        e_pad = NBUCK * g_cap
        src_p = np.zeros(e_pad, np.int64)
        dl_p = np.full(e_pad, 200.0, np.float32)
        erl_p = np.zeros(e_pad, np.int64)
        # bucket boundaries (b_c sorted)
        cnt = np.bincount(b_c, minlength=NBUCK)
        starts = np.concatenate([[0], np.cumsum(cnt)[:-1]])
        for b in range(NBUCK):
            g = cnt[b]
            o = b * g_cap
            sl = slice(starts[b], starts[b] + g)
            src_p[o:o + g] = s_c[sl]
            dl_p[o:o + g] = (dl[sl] - b * BUCKET).astype(np.float32)
            erl_p[o:o + g] = dl[sl]

        # edge slot k = (block j = k//128, partition p = k%128); SBUF array [p, j]
        goff[c] = src_p.reshape(nblk, 128).T
        eroff[c] = erl_p.reshape(nblk, 128).T
        dloc[c] = dl_p.reshape(nblk, 128).T.astype(BF16)

        xs = x[c * NPC:(c + 1) * NPC].astype(BF16)   # [6250, 256]
        xT[c, :, :NPC] = xs.T

    return wtb, goff, eroff, dloc, xT, g_cap


def _build_program(g_cap, debug=False):
    nblkb = g_cap // 128
    nblk = NBUCK * nblkb
    f32 = mybir.dt.float32
    bf16 = mybir.dt.bfloat16
    i32 = mybir.dt.int32

    nc = bass.Bass(trn_type="TRN2", num_devices=NC)
    xT_in = nc.declare_dram_parameter("xT", [IN, XT_PAD], bf16, isOutput=False)
    wtb_in = nc.declare_dram_parameter("wtb", [IN, P1COLS], bf16, isOutput=False)
    goff_in = nc.declare_dram_parameter("goff", [128, nblk], i32, isOutput=False)
    eroff_in = nc.declare_dram_parameter("eroff", [128, nblk], i32, isOutput=False)
    dloc_in = nc.declare_dram_parameter("dloc", [128, nblk], bf16, isOutput=False)
    out_ext = nc.declare_dram_parameter("out", [NPC, HC], f32, isOutput=True)
    if debug:
        dbg_tbl = nc.declare_dram_parameter("dbg_tbl", [NPC, PAY], bf16, isOutput=True)
        dbg_er = nc.declare_dram_parameter("dbg_er", [NPC, H], bf16, isOutput=True)
        dbg_g = nc.declare_dram_parameter("dbg_g", [128, PAY], bf16, isOutput=True)
        dbg_ere = nc.declare_dram_parameter("dbg_ere", [128, H], bf16, isOutput=True)
        dbg_w = nc.declare_dram_parameter("dbg_w", [128, H], bf16, isOutput=True)
        dbg_v = nc.declare_dram_parameter("dbg_v", [128, PAY], bf16, isOutput=True)
        dbg_ot = nc.declare_dram_parameter("dbg_ot", [128, 128], bf16, isOutput=True)
        dbg_ps = nc.declare_dram_parameter("dbg_ps", [128, PAY], f32, isOutput=True)
        dbg_ps1 = nc.declare_dram_parameter("dbg_ps1", [128, PAY], f32, isOutput=True)
        dbg_ps2 = nc.declare_dram_parameter("dbg_ps2", [128, PAY], f32, isOutput=True)

    tbl_loc = nc.dram_tensor("tbl_loc", [NPC, PAY], bf16)
    tbl_full = nc.dram_tensor("tbl_full", [N, PAY], bf16, addr_space="Shared")
    er_tbl = nc.dram_tensor("er_tbl", [NPC, H], bf16)

    with tile.TileContext(nc) as tc:
        # ---------------- phase 1: Wh / el / er ----------------
        with tc.tile_pool(name="p1w", bufs=1) as p1w, \
             tc.tile_pool(name="p1", bufs=3) as p1, \
             tc.tile_pool(name="ps1", bufs=2, space="PSUM") as ps1:
            xts = []
            wtbs = []
            for k in range(2):
                t = p1w.tile([128, XT_PAD], bf16, tag=f"xt{k}")
                nc.sync.dma_start(out=t[:], in_=xT_in[k * 128:(k + 1) * 128, :])
                xts.append(t)
                u = p1w.tile([128, P1COLS], bf16, tag=f"wtb{k}")
                nc.sync.dma_start(out=u[:], in_=wtb_in[k * 128:(k + 1) * 128, :])
                wtbs.append(u)
            for tn in range(NBUCK):
                ps = ps1.tile([128, P1COLS], f32)
                for k in range(2):
                    nc.tensor.matmul(
                        out=ps[:],
                        lhsT=xts[k][:, tn * 128:(tn + 1) * 128],
                        rhs=wtbs[k][:],
                        start=(k == 0), stop=(k == 1),
                    )
                sb = p1.tile([128, P1COLS], bf16)
                nc.vector.tensor_copy(out=sb[:], in_=ps[:])
                rows = min(128, NPC - tn * 128)
                nc.sync.dma_start(
                    out=tbl_loc[tn * 128:tn * 128 + rows, :], in_=sb[:rows, 0:PAY])
                nc.sync.dma_start(
                    out=er_tbl[tn * 128:tn * 128 + rows, :], in_=sb[:rows, PAY:P1COLS])

        # ---------------- all-gather the [Wh|el] table ----------------
        nc.gpsimd.collective_compute(
            "AllGather", mybir.AluOpType.bypass,
            replica_groups=[list(range(NC))],
            ins=[tbl_loc[:].opt()], outs=[tbl_full[:].opt()],
        )
        if debug:
            nc.sync.dma_start(out=dbg_tbl[:, :], in_=tbl_loc[:, :])
            nc.sync.dma_start(out=dbg_er[:, :], in_=er_tbl[:, :])

        # ---------------- phase 2: gather / score / scatter ----------------
        with tc.tile_pool(name="cst", bufs=1) as cst, \
             tc.tile_pool(name="gp", bufs=2) as gp, \
             tc.tile_pool(name="vp", bufs=24) as vp, \
             tc.tile_pool(name="otp", bufs=24) as otp, \
             tc.tile_pool(name="sp", bufs=2) as sp, \
             tc.tile_pool(name="np_", bufs=3) as np_, \
             tc.tile_pool(name="ps2", bufs=2, space="PSUM") as ps2p:

            iota_i = cst.tile([128, 128], i32)
            nc.gpsimd.iota(iota_i[:], pattern=[[1, 128]], base=0, channel_multiplier=0)
            iota_b = cst.tile([128, 128], bf16)
            nc.vector.tensor_copy(out=iota_b[:], in_=iota_i[:])

            goff_sb = cst.tile([128, nblk], i32)
            nc.sync.dma_start(out=goff_sb[:], in_=goff_in[:, :])
            eroff_sb = cst.tile([128, nblk], i32)
            nc.sync.dma_start(out=eroff_sb[:], in_=eroff_in[:, :])
            dloc_sb = cst.tile([128, nblk], bf16)
            nc.sync.dma_start(out=dloc_sb[:], in_=dloc_in[:, :])

            # HW indirect DMA consumes ONE offset per partition, streaming the
            # full per-partition output free-size contiguously (probe-verified)
            # -> one gather call per 128-edge block with [128, 1] offsets.
            er_e = cst.tile([128, nblk * H], bf16)
            er_e3 = er_e[:].rearrange("p (b h) -> p b h", h=H)
            for blk in range(nblk):
                nc.gpsimd.indirect_dma_start(
                    out=er_e3[:, blk, :], out_offset=None,
                    in_=er_tbl[:],
                    in_offset=IndirectOffsetOnAxis(
                        ap=eroff_sb[:, blk:blk + 1], axis=0),
                )

            n_sc = (NBUCK + SC_BUCKETS - 1) // SC_BUCKETS
            for sc in range(n_sc):
                b0 = sc * SC_BUCKETS
                nb_buckets = min(SC_BUCKETS, NBUCK - b0)
                nb = nb_buckets * nblkb              # blocks this super-chunk
                blk0 = b0 * nblkb

                G = gp.tile([128, nb * PAY], bf16)
                G3 = G[:].rearrange("p (b y) -> p b y", y=PAY)
                for blk in range(nb):
                    nc.gpsimd.indirect_dma_start(
                        out=G3[:, blk, :], out_offset=None,
                        in_=tbl_full[:],
                        in_offset=IndirectOffsetOnAxis(
                            ap=goff_sb[:, blk0 + blk:blk0 + blk + 1], axis=0),
                    )

                # scores: e = el + er ; leaky ; exp
                e_t = sp.tile([128, nb * H], f32, tag="e")
                e3 = e_t[:].rearrange("p (b h) -> p b h", h=H)
                nc.vector.tensor_tensor(
                    out=e3, in0=G3[:, :, HC:PAY],
                    in1=er_e3[:, blk0:blk0 + nb, :], op=mybir.AluOpType.add)
                es_t = sp.tile([128, nb * H], f32, tag="es")
                nc.vector.tensor_scalar_mul(es_t[:], e_t[:], NEG)
                nc.vector.tensor_tensor(
                    out=e_t[:], in0=e_t[:], in1=es_t[:], op=mybir.AluOpType.max)
                w_t = sp.tile([128, nb * H], bf16, tag="w")
                nc.scalar.activation(out=w_t[:], in_=e_t[:],
                                     func=mybir.ActivationFunctionType.Exp)
                w3 = w_t[:].rearrange("p (b h) -> p b h", h=H)

                # per-block V/OT tiles: matmul operands MUST be offset-0 APs —
                # a moving-operand free offset >= its inner count mis-lowers
                # (folds into the partition axis; verified on HW).
                i2 = iota_b[:]
                V_blks = []
                OT_blks = []
                for blk in range(nb):
                    Vb = vp.tile([128, PAY], bf16, tag="vblk")
                    G4b = G3[:, blk, 0:HC].rearrange("p (h c) -> p h c", c=C)
                    V4b = Vb[:, 0:HC].rearrange("p (h c) -> p h c", c=C)
                    w4b = w3[:, blk, :].to_broadcast([128, H, C])
                    nc.vector.tensor_tensor(out=V4b, in0=G4b, in1=w4b,
                                            op=mybir.AluOpType.mult)
                    nc.scalar.activation(out=Vb[:, HC:PAY], in_=w3[:, blk, :],
                                         func=mybir.ActivationFunctionType.Copy)
                    OTb = otp.tile([128, 128], bf16, tag="otblk")
                    db = dloc_sb[:, blk0 + blk].to_broadcast([128, 128])
                    nc.vector.tensor_tensor(out=OTb[:], in0=db, in1=i2,
                                            op=mybir.AluOpType.is_equal)
                    V_blks.append(Vb)
                    OT_blks.append(OTb)

                if debug and sc == 0:
                    nc.sync.dma_start(out=dbg_g[:, :], in_=G3[:, 0, :])
                    nc.sync.dma_start(out=dbg_ere[:, :], in_=er_e3[:, 0, :])
                    nc.sync.dma_start(out=dbg_w[:, :], in_=w3[:, 0, :])
                    nc.sync.dma_start(out=dbg_v[:, :], in_=V_blks[0][:, :])
                    nc.sync.dma_start(out=dbg_ot[:, :], in_=OT_blks[0][:, :])

                # scatter-accumulate per bucket, then normalize
                for bb in range(nb_buckets):
                    bucket = b0 + bb
                    ps = ps2p.tile([128, PAY], f32)
                    for j in range(nblkb):
                        blk = bb * nblkb + j
                        nc.tensor.matmul(
                            out=ps[:],
                            lhsT=OT_blks[blk][:],
                            rhs=V_blks[blk][:],
                            start=(j == 0), stop=(j == nblkb - 1),
                        )
                    if debug and bucket == 0:
                        ps_sb = np_.tile([128, PAY], f32, tag="psdump")
                        nc.vector.tensor_copy(out=ps_sb[:], in_=ps[:])
                        nc.sync.dma_start(out=dbg_ps[:, :], in_=ps_sb[:, :])
                    den = np_.tile([128, H], f32, tag="den")
                    nc.vector.tensor_scalar_add(den[:], ps[:, HC:PAY], EPS)
                    rec = np_.tile([128, H], f32, tag="rec")
                    nc.vector.reciprocal(rec[:], den[:])
                    ot = np_.tile([128, HC], f32, tag="ot")
                    ot3 = ot[:].rearrange("p (h c) -> p h c", c=C)
                    n3 = ps[:, 0:HC].rearrange("p (h c) -> p h c", c=C)
                    r3 = rec[:].to_broadcast([128, H, C])
                    nc.vector.tensor_tensor(out=ot3, in0=n3, in1=r3,
                                            op=mybir.AluOpType.mult)
                    rows = min(128, NPC - bucket * 128)
                    nc.sync.dma_start(
                        out=out_ext[bucket * 128:bucket * 128 + rows, :],
                        in_=ot[:rows, :])

    _split_excess_waits(nc)
    return nc


def kernel(**inputs):
    x = np.asarray(inputs["x"], np.float32)
    edge_index = np.asarray(inputs["edge_index"])
    W = np.asarray(inputs["W"], np.float32)
    a_left = np.asarray(inputs["a_left"], np.float32)
    a_right = np.asarray(inputs["a_right"], np.float32)

    wtb, goff, eroff, dloc, xT, g_cap = _host_prep(x, edge_index, W, a_left, a_right)
    nc = _build_program(g_cap)

    in_maps = []
    for c in range(NC):
        in_maps.append({
            "xT": np.ascontiguousarray(xT[c]),
            "wtb": wtb,
            "goff": np.ascontiguousarray(goff[c]),
            "eroff": np.ascontiguousarray(eroff[c]),
            "dloc": np.ascontiguousarray(dloc[c]),
        })

    res = run_bass_kernel_spmd(nc, in_maps, core_ids=list(range(NC)))
    out = np.concatenate([np.asarray(res.results[c]["out"]) for c in range(NC)], axis=0)
    return out.astype(np.float32)



# revision 12
# speedup vs baseline: 1.1963x; 1.1963x over previous
"""GAT layer on 8 Trainium2 NeuronCores (Bass/Tile), edge-parallel dst-sharded.

v2: all per-edge data movement via big SWDGE dma_gather calls (994ns fixed +
0.34ns/descriptor) instead of per-128-edge indirect_dma_start calls (994ns
fixed EACH -> 2.3ms of serial GPSIMD descriptor generation in v1).

Structure per core (dst-shard of 6250 nodes):
  phase 1: Wh/el/er for local nodes -> 768B-padded table rows + padded er table
  AllGather the [Wh|el] table (bf16, 384-col rows)
  phase 2, per chunk of 2 dst buckets:
    - 4 dma_gathers (one per src-range group of 12500 rows, int16-safe)
      pull [Wh|el] rows for every edge slot
    - 1 dma_gather pulls er[dst] rows from the local padded er table
    - batched DVE: scores -> leaky -> exp -> weights; one-hot build; V rows
    - per dst bucket: chained one-hot scatter matmuls in PSUM, normalize, out
"""
import sys

for _p in ("/opt/trn_rl_repo",):
    if _p not in sys.path:
        sys.path.insert(0, _p)

import numpy as np
import ml_dtypes

import concourse.bass as bass
import concourse.tile as tile
from concourse import mybir
from concourse import library_config
from concourse.bass_utils import run_bass_kernel_spmd
from concourse.library_overlay import lower_extended_insts

BF16 = ml_dtypes.bfloat16

N = 50000
E = 800000
IN = 256
H = 8
C = 32
HC = H * C            # 256
NC = 8
NPC = N // NC         # 6250 nodes per core
BUCKET = 128
NBUCK = (NPC + BUCKET - 1) // BUCKET   # 49
XT_PAD = NBUCK * 128                   # 6272
P1COLS = HC + 2 * H   # 272: phase-1 matmul out [Wh | el | er]
ROW = 384             # padded table row (768 B, %256)
ERROW = 128           # padded er row (256 B)
NGROUP = 2
GRPR = N // NGROUP    # 25000 table rows per group section (int16-safe < 32768)
CHUNKB = 2            # dst buckets per phase-2 chunk
MAXIDX = 1024         # HW cap on num_idxs per dma_gather call (probe-verified)
NEG = 0.2
EPS = 1e-16

# walrus in this container caps sync waits per instruction at 1; hoist excess
# onto same-engine NoOps.
_waitfix_ctr = [0]


def _split_excess_waits(nc, max_waits=1):
    n_fixed = 0
    for fn in nc.m.functions:
        for bb in fn.blocks:
            insts = bb.instructions
            out = []
            for ins in insts:
                si = ins.sync_info
                waits = list(si.on_wait) if si is not None and si.on_wait else []
                if len(waits) > max_waits:
                    keep = waits[-max_waits:]
                    extra = waits[:-max_waits]
                    for i in range(0, len(extra), max_waits):
                        grp = extra[i:i + max_waits]
                        _waitfix_ctr[0] += 1
                        nop = mybir.InstNoOp(
                            name=f"I-waitfix-{_waitfix_ctr[0]}", ins=[], outs=[])
                        nop.engine = ins.engine
                        nop.sync_info = mybir.SyncInfo(on_wait=grp, on_update=[])
                        nc.register_instruction(nop)
                        out.append(nop)
                    si.on_wait = keep
                    n_fixed += 1
                out.append(ins)
            if len(out) != len(insts):
                bb.instructions = out
    return n_fixed


class Plan:
    """Compiled-in slot layout, identical across cores (SPMD)."""

    def __init__(self, caps):
        # caps[b][g] = blocks for (bucket b, group g), uniform across cores
        self.caps = caps
        self.nchunk = (NBUCK + CHUNKB - 1) // CHUNKB
        self.chunks = []  # per chunk: dict with slot ranges
        blk = 0
        for ci in range(self.nchunk):
            buckets = list(range(ci * CHUNKB, min((ci + 1) * CHUNKB, NBUCK)))
            grp_off = []       # block offset (within chunk) of each group run
            grp_nb = []        # blocks in each group run
            bucket_blocks = {b: [] for b in buckets}
            off = 0
            for g in range(NGROUP):
                grp_off.append(off)
                nbg = 0
                for b in buckets:
                    nblk_bg = caps[b][g]
                    bucket_blocks[b].extend(range(off + nbg, off + nbg + nblk_bg))
                    nbg += nblk_bg
                grp_nb.append(nbg)
                off += nbg
            self.chunks.append({
                "buckets": buckets,
                "grp_off": grp_off,
                "grp_nb": grp_nb,
                "nb": off,
                "blk0": blk,
            })
            blk += off
        self.nblk = blk
        self.maxnb = max(c["nb"] for c in self.chunks)


def _host_prep(x, edge_index, W, a_left, a_right):
    src = np.concatenate([np.asarray(edge_index[0], np.int64),
                          np.arange(N, dtype=np.int64)])
    dst = np.concatenate([np.asarray(edge_index[1], np.int64),
                          np.arange(N, dtype=np.int64)])

    # fold attention vectors through W:  [el|er] = x @ (W.T @ A)
    A = np.zeros((HC, 2 * H), np.float32)
    for h in range(H):
        A[h * C:(h + 1) * C, h] = a_left[h]
        A[h * C:(h + 1) * C, H + h] = a_right[h]
    B = (W.T.astype(np.float64) @ A.astype(np.float64)).astype(np.float32)
    wtb = np.concatenate([W.T.astype(np.float32), B], axis=1).astype(BF16)  # [256, 272]

    core = dst // NPC

    # per-core, per-(bucket, group) counts -> uniform caps
    counts = np.zeros((NC, NBUCK, NGROUP), np.int64)
    per_core = []
    for c in range(NC):
        m = core == c
        s_c, d_c = src[m], dst[m]
        dl = d_c - c * NPC
        b_c = dl // BUCKET
        g_c = s_c // GRPR
        np.add.at(counts[c], (b_c, g_c), 1)
        per_core.append((s_c, dl, b_c, g_c))
    caps = (counts.max(axis=0) + 127) // 128   # [NBUCK, NGROUP] blocks
    plan = Plan(caps.tolist())

    nblk = plan.nblk
    nslot = nblk * 128
    goff = np.zeros((NC, nslot), np.int16)
    eroff = np.zeros((NC, nslot), np.int16)
    dlocv = np.full((NC, nslot), 200.0, np.float32)

    for c in range(NC):
        s_c, dl, b_c, g_c = per_core[c]
        # order edges by (chunk, group, bucket, src) to match slot layout
        chunk_c = b_c // CHUNKB
        order = np.lexsort((s_c, b_c, g_c, chunk_c))
        s_c, dl, b_c, g_c = s_c[order], dl[order], b_c[order], g_c[order]
        # region start slot for each (b, g)
        pos = 0
        region_start = {}
        for ch in plan.chunks:
            for g in range(NGROUP):
                for b in ch["buckets"]:
                    region_start[(b, g)] = (ch["blk0"] + ch["grp_off"][g]) * 128 \
                        + sum(plan.caps[b2][g] for b2 in ch["buckets"] if b2 < b) * 128
        # fill slots; edges sorted so each region's edges are contiguous
        cnt = np.zeros((NBUCK, NGROUP), np.int64)
        idx_sorted = np.ravel_multi_index((b_c, g_c), (NBUCK, NGROUP))
        # compute slot for each edge: region_start + running count
        starts = np.array([[region_start[(b, g)] for g in range(NGROUP)]
                           for b in range(NBUCK)], np.int64)
        # running position within each region
        run = np.zeros(len(s_c), np.int64)
        uniq, first_pos, inv_counts = np.unique(idx_sorted, return_index=True,
                                                return_counts=True)
        for u, fp, ct in zip(uniq, first_pos, inv_counts):
            run[fp:fp + ct] = np.arange(ct)
        slots = starts[b_c, g_c] + run
        goff[c][slots] = (s_c - g_c * GRPR).astype(np.int16)
        eroff[c][slots] = dl.astype(np.int16)
        dlocv[c][slots] = (dl - b_c * BUCKET).astype(np.float32)

    # wrapped int16 idx layout: slot s -> [s%16, s//16], replicated x8
    def wrap(vals):
        a = vals.reshape(nslot // 16, 16).T           # [16, S/16]
        return np.tile(a, (8, 1)).copy()              # [128, S/16]

    idx_main = np.stack([wrap(goff[c]) for c in range(NC)])
    idx_er = np.stack([wrap(eroff[c]) for c in range(NC)])
    # dloc: slot s -> [s%128, s//128]
    dloc = np.stack([dlocv[c].reshape(nblk, 128).T.astype(BF16)
                     for c in range(NC)])

    xT = np.zeros((NC, IN, XT_PAD), BF16)
    for c in range(NC):
        xT[c, :, :NPC] = x[c * NPC:(c + 1) * NPC].astype(BF16).T

    iota = np.tile(np.arange(128, dtype=np.float32)[None, :],
                   (128, plan.maxnb)).astype(BF16)    # [128, maxnb*128]

    return plan, wtb, idx_main, idx_er, dloc, xT, iota


def _build_program(plan):
    f32 = mybir.dt.float32
    bf16 = mybir.dt.bfloat16
    i16 = mybir.dt.int16
    nblk = plan.nblk

    nc = bass.Bass(trn_type="TRN2", num_devices=NC)
    xT_in = nc.declare_dram_parameter("xT", [IN, XT_PAD], bf16, isOutput=False)
    wtb_in = nc.declare_dram_parameter("wtb", [IN, P1COLS], bf16, isOutput=False)
    idxm_in = nc.declare_dram_parameter("idxm", [128, nblk * 8], i16, isOutput=False)
    idxe_in = nc.declare_dram_parameter("idxe", [128, nblk * 8], i16, isOutput=False)
    dloc_in = nc.declare_dram_parameter("dloc", [128, nblk], bf16, isOutput=False)
    iota_in = nc.declare_dram_parameter("iota", [128, plan.maxnb * 128], bf16,
                                        isOutput=False)
    out_ext = nc.declare_dram_parameter("out", [NPC, HC], f32, isOutput=True)

    tbl_loc = nc.dram_tensor("tbl_loc", [NPC, ROW], bf16)
    tbl_full = nc.dram_tensor("tbl_full", [N, ROW], bf16, addr_space="Shared")
    er_pad = nc.dram_tensor("er_pad", [NPC, ERROW], bf16)

    # one Pool register per distinct num_idxs value (to_reg per call exhausts
    # the register file)
    _regs = {}

    def nreg(v):
        if v not in _regs:
            _regs[v] = nc.gpsimd.to_reg(v)
        return _regs[v]

    with tile.TileContext(nc) as tc:
        nc.gpsimd.load_library(library_config.mlp)

        # ---------------- phase 1: Wh / el / er ----------------
        with tc.tile_pool(name="p1w", bufs=1) as p1w, \
             tc.tile_pool(name="p1", bufs=3) as p1, \
             tc.tile_pool(name="ps1", bufs=2, space="PSUM") as ps1:
            xts = []
            wtbs = []
            for k in range(2):
                t = p1w.tile([128, XT_PAD], bf16, tag=f"xt{k}")
                nc.sync.dma_start(out=t[:], in_=xT_in[k * 128:(k + 1) * 128, :])
                xts.append(t)
                u = p1w.tile([128, P1COLS], bf16, tag=f"wtb{k}")
                nc.sync.dma_start(out=u[:], in_=wtb_in[k * 128:(k + 1) * 128, :])
                wtbs.append(u)
            for tn in range(NBUCK):
                ps = ps1.tile([128, P1COLS], f32)
                for k in range(2):
                    nc.tensor.matmul(
                        out=ps[:],
                        lhsT=xts[k][:, tn * 128:(tn + 1) * 128],
                        rhs=wtbs[k][:],
                        start=(k == 0), stop=(k == 1),
                    )
                sb = p1.tile([128, P1COLS], bf16)
                nc.vector.tensor_copy(out=sb[:], in_=ps[:])
                rows = min(128, NPC - tn * 128)
                nc.sync.dma_start(
                    out=tbl_loc[tn * 128:tn * 128 + rows, 0:P1COLS],
                    in_=sb[:rows, :])
                nc.sync.dma_start(
                    out=er_pad[tn * 128:tn * 128 + rows, 0:H],
                    in_=sb[:rows, HC + H:P1COLS])

        # ---------------- all-gather the padded table ----------------
        nc.gpsimd.collective_compute(
            "AllGather", mybir.AluOpType.bypass,
            replica_groups=[list(range(NC))],
            ins=[tbl_loc[:].opt()], outs=[tbl_full[:].opt()],
        )

        # ---------------- phase 2: gather / score / scatter ----------------
        with tc.tile_pool(name="cst", bufs=1) as cst, \
             tc.tile_pool(name="gp", bufs=2) as gp, \
             tc.tile_pool(name="erp", bufs=2) as erp, \
             tc.tile_pool(name="wp", bufs=2) as wp, \
             tc.tile_pool(name="np_", bufs=3) as np_, \
             tc.tile_pool(name="ps2", bufs=4, space="PSUM") as ps2p:

            iota_sb = cst.tile([128, plan.maxnb * 128], bf16)
            nc.sync.dma_start(out=iota_sb[:], in_=iota_in[:, :])
            dloc_sb = cst.tile([128, nblk], bf16)
            nc.sync.dma_start(out=dloc_sb[:], in_=dloc_in[:, :])
            idxm_sb = cst.tile([128, nblk * 8], i16)
            nc.sync.dma_start(out=idxm_sb[:], in_=idxm_in[:, :])
            idxe_sb = cst.tile([128, nblk * 8], i16)
            nc.sync.dma_start(out=idxe_sb[:], in_=idxe_in[:, :])

            for ch in plan.chunks:
                nb = ch["nb"]
                blk0 = ch["blk0"]
                G = gp.tile([128, nb, ROW], bf16, tag="G")
                maxb = MAXIDX // 128
                for g in range(NGROUP):
                    nbg = ch["grp_nb"][g]
                    for sub in range(0, nbg, maxb):
                        nsub = min(maxb, nbg - sub)
                        boff = ch["grp_off"][g] + sub
                        s0 = (blk0 + boff) * 128
                        nidx = nsub * 128
                        nc.gpsimd.dma_gather(
                            G[:, boff:boff + nsub, :],
                            tbl_full[g * GRPR:(g + 1) * GRPR, :],
                            idxm_sb[:, s0 // 16:(s0 + nidx) // 16],
                            nidx, nreg(nidx), ROW)
                ER = erp.tile([128, nb, ERROW], bf16, tag="ER")
                for sub in range(0, nb, maxb):
                    nsub = min(maxb, nb - sub)
                    s0 = (blk0 + sub) * 128
                    nidx = nsub * 128
                    nc.gpsimd.dma_gather(
                        ER[:, sub:sub + nsub, :], er_pad[:],
                        idxe_sb[:, s0 // 16:(s0 + nidx) // 16],
                        nidx, nreg(nidx), ERROW)

                # scores: e = el + er ; leaky ; exp
                e_t = wp.tile([128, nb, H], f32, tag="e")
                nc.vector.tensor_tensor(
                    out=e_t[:], in0=G[:, :, HC:HC + H], in1=ER[:, :, 0:H],
                    op=mybir.AluOpType.add)
                es_t = wp.tile([128, nb * H], f32, tag="es")
                e2 = e_t[:].rearrange("p b h -> p (b h)")
                nc.vector.tensor_scalar_mul(es_t[:], e2, NEG)
                nc.vector.tensor_tensor(
                    out=e2, in0=e2, in1=es_t[:], op=mybir.AluOpType.max)
                w_t = wp.tile([128, nb, H], bf16, tag="w")
                nc.scalar.activation(out=w_t[:], in_=e_t[:],
                                     func=mybir.ActivationFunctionType.Exp)

                # one-hot (edges x dst-in-bucket) for the whole chunk
                OT = wp.tile([128, nb * 128], bf16, tag="OT")
                OT3 = OT[:].rearrange("p (b x) -> p b x", x=128)
                dloc3 = dloc_sb[:, blk0:blk0 + nb].to_broadcast([128, nb, 128])
                iota3 = iota_sb[:, 0:nb * 128].rearrange(
                    "p (b x) -> p b x", x=128)
                nc.vector.tensor_tensor(out=OT3, in0=dloc3, in1=iota3,
                                        op=mybir.AluOpType.is_equal)

                # V rows: [w * Wh | w]
                V = wp.tile([128, nb, HC + H], bf16, tag="V")
                G4 = G[:, :, 0:HC].rearrange("p b (h c) -> p b h c", c=C)
                V4 = V[:, :, 0:HC].rearrange("p b (h c) -> p b h c", c=C)
                w4 = w_t[:].to_broadcast([128, nb, H, C])
                nc.vector.tensor_tensor(out=V4, in0=G4, in1=w4,
                                        op=mybir.AluOpType.mult)
                nc.scalar.activation(out=V[:, :, HC:HC + H], in_=w_t[:],
                                     func=mybir.ActivationFunctionType.Copy)

                OT2 = OT[:]
                V2 = V[:].rearrange("p b y -> p (b y)")
                for b in ch["buckets"]:
                    ps = ps2p.tile([128, HC + H], f32)
                    blocks = _bucket_blocks(plan, ch, b)
                    for j, blk in enumerate(blocks):
                        nc.tensor.matmul(
                            out=ps[:],
                            lhsT=OT2[:, blk * 128:(blk + 1) * 128],
                            rhs=V2[:, blk * (HC + H):(blk + 1) * (HC + H)],
                            start=(j == 0), stop=(j == len(blocks) - 1),
                        )
                    den = np_.tile([128, H], f32, tag="den")
                    nc.vector.tensor_scalar_add(den[:], ps[:, HC:HC + H], EPS)
                    rec = np_.tile([128, H], f32, tag="rec")
                    nc.vector.reciprocal(rec[:], den[:])
                    ot = np_.tile([128, HC], f32, tag="ot")
                    ot3 = ot[:].rearrange("p (h c) -> p h c", c=C)
                    n3 = ps[:, 0:HC].rearrange("p (h c) -> p h c", c=C)
                    r3 = rec[:].to_broadcast([128, H, C])
                    nc.vector.tensor_tensor(out=ot3, in0=n3, in1=r3,
                                            op=mybir.AluOpType.mult)
                    rows = min(128, NPC - b * 128)
                    nc.sync.dma_start(
                        out=out_ext[b * 128:b * 128 + rows, :],
                        in_=ot[:rows, :])

    lower_extended_insts(nc)
    _split_excess_waits(nc)
    return nc


def _bucket_blocks(plan, ch, b):
    """Block positions (within chunk) belonging to bucket b."""
    blocks = []
    for g in range(NGROUP):
        off = ch["grp_off"][g]
        for b2 in ch["buckets"]:
            nbb = plan.caps[b2][g]
            if b2 == b:
                blocks.extend(range(off, off + nbb))
            off += nbb
    return blocks


def kernel(**inputs):
    x = np.asarray(inputs["x"], np.float32)
    edge_index = np.asarray(inputs["edge_index"])
    W = np.asarray(inputs["W"], np.float32)
    a_left = np.asarray(inputs["a_left"], np.float32)
    a_right = np.asarray(inputs["a_right"], np.float32)

    plan, wtb, idx_main, idx_er, dloc, xT, iota = _host_prep(
        x, edge_index, W, a_left, a_right)
    nc = _build_program(plan)

    in_maps = []
    for c in range(NC):
        in_maps.append({
            "xT": np.ascontiguousarray(xT[c]),
            "wtb": wtb,
            "idxm": np.ascontiguousarray(idx_main[c]),
            "idxe": np.ascontiguousarray(idx_er[c]),
            "dloc": np.ascontiguousarray(dloc[c]),
            "iota": iota,
        })

    res = run_bass_kernel_spmd(nc, in_maps, core_ids=list(range(NC)))
    out = np.concatenate([np.asarray(res.results[c]["out"]) for c in range(NC)], axis=0)
    return out.astype(np.float32)
